# revision 1
# baseline (speedup 1.0000x reference)
"""Conv3d (k=3, pad=1) as shifted-window matmuls on 8 TRN2 NeuronCores.

Sharding: data-parallel over (batch B=2) x (T quarters of 8 output frames).
Each core computes out[b, :, t0:t0+8, :, :] from a host-padded input shard
xs[ci, 10, 130, 130] (conv zero-padding + t-halo baked in by the host).

Per-core formulation: output tile M=128 = (co=32, dt=2, dh=2) output
positions, contraction K=128 = (jt=4 t-window slots, jhg=2 h-parity, ci=16),
N=512 = (4 h-blocks x 128 w). The 3x3x3 kernel becomes 6 accumulating
matmuls (kw=3 x jhh=2) per PSUM tile, with all w/h shifts expressed as free-
dim AP offsets into one SBUF-resident tile. float32r gives full-rate PE
matmuls (FP22 multiply, fp32 accumulate).
"""

import sys

if "/opt/trn_rl_repo" not in sys.path:
    sys.path.insert(0, "/opt/trn_rl_repo")

import numpy as np

import concourse.bass as bass
import concourse.mybir as mybir
import concourse.tile as tile
from concourse.bass_utils import run_bass_kernel_spmd

B, C_IN, T, H, W = 2, 16, 32, 128, 128
C_OUT, KS = 32, 3
N_CORES = 8
TSH = T // 4          # output frames per core
NBT = TSH // 2        # bt tiles per core (2 output frames each)
HB = H // 2           # h blocks (dh=2)
NNB = HB // 4         # 512-wide n-blocks per bt tile (4 h-blocks x 128 w)


def _split_excess_waits(nc, limit=1):
    """This walrus build accepts at most ONE sync-wait command per
    instruction. Move excess waits onto same-engine single-wait NoOps placed
    immediately before the instruction (identical blocking semantics)."""
    uid = 0
    for f in nc.m.functions:
        for bb in f.blocks:
            out = []
            for inst in bb.instructions:
                si = inst.sync_info
                if si is not None and si.on_wait and len(si.on_wait) > limit:
                    waits = list(si.on_wait)
                    excess, keep = waits[:-limit], waits[-limit:]
                    for k in range(0, len(excess), limit):
                        nop = mybir.InstNoOp(
                            name=f"wait_split_{uid}", ins=[], outs=[],
                            sync_info=mybir.SyncInfo(
                                on_wait=list(excess[k:k + limit]), on_update=[]))
                        nop.engine = inst.engine
                        nc.register_instruction(nop)
                        uid += 1
                        out.append(nop)
                    si.on_wait = keep
                out.append(inst)
            bb.instructions[:] = out
    return nc


def _build_program(split=True, repeat=1):
    nc = bass.Bass()
    f32 = mybir.dt.float32
    f32r = mybir.dt.float32r
    # Host pre-arranges the shard partition-major so every tile load/store is
    # ONE <=3-dim DMA: xs[f, jhg, ci, bh, w], out[bt, dt, dh, co, bh, w].
    xs = nc.dram_tensor("xs", [TSH + 2, 2, C_IN, HB + 1, W + 2], f32,
                        kind="ExternalInput")
    wt = nc.dram_tensor("wt", [6, 128, 128], f32, kind="ExternalInput")
    bi = nc.dram_tensor("bi", [128, 1], f32, kind="ExternalInput")
    out = nc.dram_tensor("out", [NBT, 2, 2, C_OUT, HB, W], f32,
                         kind="ExternalOutput")

    with tile.TileContext(nc) as tc:
        with tc.tile_pool(name="wpool", bufs=1) as wpool, \
             tc.tile_pool(name="xpool", bufs=2) as xpool, \
             tc.tile_pool(name="opool", bufs=2) as opool, \
             tc.tile_pool(name="pspool", bufs=2, space="PSUM") as pspool:
            w_sb = wpool.tile([128, 6, 128], f32r)
            nc.sync.dma_start(out=w_sb[:, :, :],
                              in_=wt.rearrange("i p m -> p i m").bitcast(f32r))
            b_sb = wpool.tile([128, 1], f32)
            nc.sync.dma_start(out=b_sb[:, :], in_=bi[:, :])

            import contextlib
            rep_ctx = (tc.For_i(0, repeat, 1,
                                hint_engines=(mybir.EngineType.PE,
                                              mybir.EngineType.SP,
                                              mybir.EngineType.DVE))
                       if repeat > 1 else contextlib.nullcontext())
            with rep_ctx:
                body(nc, tc, xs, w_sb, b_sb, out, xpool, opool, pspool)
    if split:
        _split_excess_waits(nc)
    return nc


def body(nc, tc, xs, w_sb, b_sb, out, xpool, opool, pspool):
    f32 = mybir.dt.float32
    f32r = mybir.dt.float32r
    if True:
            for bt in range(NBT):
                x_t = xpool.tile([128, HB + 1, W + 2], f32r, name="x_t")
                src = xs[2 * bt:2 * bt + 4].rearrange(
                    "f j c b w -> (f j c) b w").bitcast(f32r)
                nc.sync.dma_start(out=x_t[:, :, :], in_=src)

                out_t = opool.tile([128, HB, W], f32, name="out_t")
                for g in range(NNB // 4):
                    pss = [pspool.tile([128, 4, W], f32, name=f"ps{j}")
                           for j in range(4)]
                    for i in range(6):
                        kw, jhh = divmod(i, 2)
                        lhsT = w_sb[:, i, :]
                        for j in range(4):
                            nb = g * 4 + j
                            rhs = x_t[:, 4 * nb + jhh:4 * nb + jhh + 4,
                                      kw:kw + W]
                            nc.tensor.matmul(pss[j][:, :, :], lhsT, rhs,
                                             start=(i == 0), stop=(i == 5))
                    for j in range(4):
                        nb = g * 4 + j
                        nc.vector.tensor_scalar_add(
                            out_t[:, 4 * nb:4 * nb + 4, :],
                            pss[j][:, :, :], b_sb[:, 0:1])
                dst = out[bt].rearrange("dt dh co b w -> (dt dh co) b w")
                nc.sync.dma_start(out=dst, in_=out_t[:, :, :])


_NC_CACHE = []


def _get_nc():
    if not _NC_CACHE:
        _NC_CACHE.append(_build_program())
    return _NC_CACHE[0]


def _pack_weights(weight):
    wt = np.zeros((6, 128, 128), np.float32)
    for kw in range(3):
        for jhh in range(2):
            i = kw * 2 + jhh
            for jt in range(4):
                for jhg in range(2):
                    jh = 2 * jhh + jhg
                    r0 = jt * 32 + jhg * 16
                    for dt in range(2):
                        kt = jt - dt
                        if not 0 <= kt < KS:
                            continue
                        for dh in range(2):
                            kh = jh - dh
                            if not 0 <= kh < KS:
                                continue
                            c0 = dt * 64 + dh * 32
                            wt[i, r0:r0 + 16, c0:c0 + 32] = \
                                weight[:, :, kt, kh, kw].T
    return wt


def run(x, weight, bias, trace=False):
    x = np.asarray(x, dtype=np.float32)
    weight = np.asarray(weight, dtype=np.float32)
    bias = np.asarray(bias, dtype=np.float32)

    xp = np.zeros((B, C_IN, T + 2, H + 2, W + 2), np.float32)
    xp[:, :, 1:-1, 1:-1, 1:-1] = x
    wt = _pack_weights(weight)
    bi = np.tile(bias, 4).reshape(128, 1).astype(np.float32)

    in_maps = []
    for c in range(N_CORES):
        b, q = divmod(c, 4)
        t0 = q * TSH
        sh = xp[b, :, t0:t0 + TSH + 2]                # [ci, f, 130, 130]
        sh = sh.reshape(C_IN, TSH + 2, HB + 1, 2, W + 2)
        sh = np.ascontiguousarray(sh.transpose(1, 3, 0, 2, 4))
        in_maps.append({"xs": sh, "wt": wt, "bi": bi})

    nc = _get_nc()
    res = run_bass_kernel_spmd(nc, in_maps, list(range(N_CORES)), trace=trace)

    outp = np.empty((B, C_OUT, T, H, W), np.float32)
    for c in range(N_CORES):
        b, q = divmod(c, 4)
        r = res.results[c]["out"]                     # [bt, dt, dh, co, bh, w]
        r = r.transpose(3, 0, 1, 4, 2, 5).reshape(C_OUT, TSH, H, W)
        outp[b, :, q * TSH:(q + 1) * TSH] = r
    return outp, res


def kernel(x, weight, bias):
    outp, _ = run(x, weight, bias, trace=False)
    return outp



# revision 2
# speedup vs baseline: 1.2719x; 1.2719x over previous
"""Conv3d (k=3, pad=1) as shifted-window matmuls on 8 TRN2 NeuronCores.

Sharding: data-parallel over (batch B=2) x (T quarters of 8 output frames).
Each core computes out[b, :, t0:t0+8, :, :] from a host-padded input shard
xs[f, jhg, ci, bh, w] (conv zero-padding + t-halo baked in by the host).

Per-core formulation: output tile M=128 = (co=32, dt=2, dh=2) output
positions, contraction K=128 = (jt=4 t-window slots, jhg=2 h-parity, ci=16),
N=512 = (4 h-blocks x 128 w). The 3x3x3 kernel becomes 6 accumulating
matmuls (kw=3 x jhh=2) per PSUM tile, with all w/h shifts expressed as free-
dim AP offsets into one SBUF-resident tile.

v2: bf16 operands end-to-end on the device. bf16 weights enable the PE's
fast-weight-load path (fp32r LDWEIGHTS cost ~195 ns/matmul in the v1
trace); bf16 halves both input and output HBM traffic. PSUM accumulation
stays fp32; host casts the bf16 output back to fp32. Input loads are split
into 4 h-chunks per t-tile so the first matmuls start ~4x earlier, and
outputs are written per 16-row group to shorten the drain tail.
"""

import sys

if "/opt/trn_rl_repo" not in sys.path:
    sys.path.insert(0, "/opt/trn_rl_repo")

import numpy as np
import ml_dtypes

import concourse.bass as bass
import concourse.mybir as mybir
import concourse.tile as tile
from concourse.bass_utils import run_bass_kernel_spmd

B, C_IN, T, H, W = 2, 16, 32, 128, 128
C_OUT, KS = 32, 3
N_CORES = 8
TSH = T // 4          # output frames per core
NBT = TSH // 2        # bt tiles per core (2 output frames each)
HB = H // 2           # h blocks (dh=2)
NG = 4                # h-chunk groups per bt tile (16 h-blocks each)
GB = HB // NG         # h-blocks per group


def _split_excess_waits(nc, limit=1):
    """This walrus build accepts at most ONE sync-wait command per
    instruction. Move excess waits onto same-engine single-wait NoOps placed
    immediately before the instruction (identical blocking semantics)."""
    uid = 0
    for f in nc.m.functions:
        for bb in f.blocks:
            out = []
            for inst in bb.instructions:
                si = inst.sync_info
                if si is not None and si.on_wait and len(si.on_wait) > limit:
                    waits = list(si.on_wait)
                    excess, keep = waits[:-limit], waits[-limit:]
                    for k in range(0, len(excess), limit):
                        nop = mybir.InstNoOp(
                            name=f"wait_split_{uid}", ins=[], outs=[],
                            sync_info=mybir.SyncInfo(
                                on_wait=list(excess[k:k + limit]), on_update=[]))
                        nop.engine = inst.engine
                        nc.register_instruction(nop)
                        uid += 1
                        out.append(nop)
                    si.on_wait = keep
                out.append(inst)
            bb.instructions[:] = out
    return nc


def _build_program(split=True):
    nc = bass.Bass()
    f32 = mybir.dt.float32
    bf16 = mybir.dt.bfloat16
    # Host pre-arranges the shard partition-major so every tile load/store is
    # ONE <=3-dim DMA: xs[f, jhg, ci, bh, w], out[bt, dt, dh, co, bh, w].
    xs = nc.dram_tensor("xs", [TSH + 2, 2, C_IN, HB + 1, W + 2], bf16,
                        kind="ExternalInput")
    wt = nc.dram_tensor("wt", [6, 128, 128], bf16, kind="ExternalInput")
    bi = nc.dram_tensor("bi", [128, 1], f32, kind="ExternalInput")
    out = nc.dram_tensor("out", [NBT, 2, 2, C_OUT, HB, W], bf16,
                         kind="ExternalOutput")

    with tile.TileContext(nc) as tc:
        with tc.tile_pool(name="wpool", bufs=1) as wpool, \
             tc.tile_pool(name="xpool", bufs=2) as xpool, \
             tc.tile_pool(name="opool", bufs=3) as opool, \
             tc.tile_pool(name="pspool", bufs=2, space="PSUM") as pspool:
            w_sb = wpool.tile([128, 6, 128], bf16)
            nc.sync.dma_start(out=w_sb[:, :, :],
                              in_=wt.rearrange("i p m -> p i m"))
            b_sb = wpool.tile([128, 1], f32)
            nc.sync.dma_start(out=b_sb[:, :], in_=bi[:, :])

            for bt in range(NBT):
                # 4 h-chunks, each GB+1=17 rows (1-row overlap at the top):
                # chunk g covers h-blocks [16g, 16g+16] so group g's matmuls
                # depend only on chunk g.
                x_g = [xpool.tile([128, GB + 1, W + 2], bf16, name=f"x{g}")
                       for g in range(NG)]
                src = xs[2 * bt:2 * bt + 4].rearrange("f j c b w -> (f j c) b w")
                for g in range(NG):
                    nc.sync.dma_start(
                        out=x_g[g][:, :, :],
                        in_=src[:, GB * g:GB * g + GB + 1, :])

                for g in range(NG):
                    pss = [pspool.tile([128, 4, W], f32, name=f"ps{j}")
                           for j in range(4)]
                    for i in range(6):
                        kw, jhh = divmod(i, 2)
                        lhsT = w_sb[:, i, :]
                        for j in range(4):
                            rhs = x_g[g][:, 4 * j + jhh:4 * j + jhh + 4,
                                         kw:kw + W]
                            nc.tensor.matmul(pss[j][:, :, :], lhsT, rhs,
                                             start=(i == 0), stop=(i == 5))
                    out_g = opool.tile([128, GB, W], bf16, name="out_g")
                    for j in range(4):
                        nc.vector.tensor_scalar_add(
                            out_g[:, 4 * j:4 * j + 4, :],
                            pss[j][:, :, :], b_sb[:, 0:1])
                    dst = out[bt].rearrange("dt dh co b w -> (dt dh co) b w")
                    nc.sync.dma_start(out=dst[:, GB * g:GB * (g + 1), :],
                                      in_=out_g[:, :, :])
    if split:
        _split_excess_waits(nc)
    return nc


_NC_CACHE = []


def _get_nc():
    if not _NC_CACHE:
        _NC_CACHE.append(_build_program())
    return _NC_CACHE[0]


def _pack_weights(weight):
    wt = np.zeros((6, 128, 128), np.float32)
    for kw in range(3):
        for jhh in range(2):
            i = kw * 2 + jhh
            for jt in range(4):
                for jhg in range(2):
                    jh = 2 * jhh + jhg
                    r0 = jt * 32 + jhg * 16
                    for dt in range(2):
                        kt = jt - dt
                        if not 0 <= kt < KS:
                            continue
                        for dh in range(2):
                            kh = jh - dh
                            if not 0 <= kh < KS:
                                continue
                            c0 = dt * 64 + dh * 32
                            wt[i, r0:r0 + 16, c0:c0 + 32] = \
                                weight[:, :, kt, kh, kw].T
    return wt.astype(ml_dtypes.bfloat16)


def run(x, weight, bias, trace=False):
    x = np.asarray(x, dtype=np.float32)
    weight = np.asarray(weight, dtype=np.float32)
    bias = np.asarray(bias, dtype=np.float32)

    xp = np.zeros((B, C_IN, T + 2, H + 2, W + 2), ml_dtypes.bfloat16)
    xp[:, :, 1:-1, 1:-1, 1:-1] = x.astype(ml_dtypes.bfloat16)
    wt = _pack_weights(weight)
    bi = np.tile(bias, 4).reshape(128, 1).astype(np.float32)

    in_maps = []
    for c in range(N_CORES):
        b, q = divmod(c, 4)
        t0 = q * TSH
        sh = xp[b, :, t0:t0 + TSH + 2]                # [ci, f, 130, 130]
        sh = sh.reshape(C_IN, TSH + 2, HB + 1, 2, W + 2)
        sh = np.ascontiguousarray(sh.transpose(1, 3, 0, 2, 4))
        in_maps.append({"xs": sh, "wt": wt, "bi": bi})

    nc = _get_nc()
    res = run_bass_kernel_spmd(nc, in_maps, list(range(N_CORES)), trace=trace)

    outp = np.empty((B, C_OUT, T, H, W), np.float32)
    for c in range(N_CORES):
        b, q = divmod(c, 4)
        r = res.results[c]["out"]                     # [bt, dt, dh, co, bh, w]
        r = np.asarray(r).astype(np.float32)
        r = r.transpose(3, 0, 1, 4, 2, 5).reshape(C_OUT, TSH, H, W)
        outp[b, :, q * TSH:(q + 1) * TSH] = r
    return outp, res


def kernel(x, weight, bias):
    outp, _ = run(x, weight, bias, trace=False)
    return outp


# revision 5
# speedup vs baseline: 1.2913x; 1.0153x over previous
"""Conv3d (k=3, pad=1) as shifted-window matmuls on 8 TRN2 NeuronCores.

Sharding: data-parallel over (batch B=2) x (T quarters of 8 output frames).
Each core computes out[b, :, t0:t0+8, :, :] from a host-padded input shard
xs[f, jhg, ci, bh, w] (conv zero-padding + t-halo baked in by the host).

Per-core formulation: output tile M=128 = (co=32, dt=2, dh=2) output
positions, contraction K=128 = (jt=4 t-window slots, jhg=2 h-parity, ci=16),
N=512 = (4 h-blocks x 128 w). The 3x3x3 kernel becomes 6 accumulating
matmuls (kw=3 x jhh=2) per PSUM tile, with all w/h shifts expressed as free-
dim AP offsets into one SBUF-resident tile.

v2: bf16 operands end-to-end on the device. bf16 weights enable the PE's
fast-weight-load path (fp32r LDWEIGHTS cost ~195 ns/matmul in the v1
trace); bf16 halves both input and output HBM traffic. PSUM accumulation
stays fp32; host casts the bf16 output back to fp32. Input loads are split
into 4 h-chunks per t-tile so the first matmuls start ~4x earlier, and
outputs are written per 16-row group to shorten the drain tail.
"""

import sys

if "/opt/trn_rl_repo" not in sys.path:
    sys.path.insert(0, "/opt/trn_rl_repo")

import numpy as np
import ml_dtypes

import concourse.bass as bass
import concourse.mybir as mybir
import concourse.tile as tile
from concourse.bass_utils import run_bass_kernel_spmd

B, C_IN, T, H, W = 2, 16, 32, 128, 128
C_OUT, KS = 32, 3
N_CORES = 8
TSH = T // 4          # output frames per core
NBT = TSH // 2        # bt tiles per core (2 output frames each)
HB = H // 2           # h blocks (dh=2)
NG = 4                # h-chunk groups per bt tile (16 h-blocks each)
GB = HB // NG         # h-blocks per group


def _split_excess_waits(nc, limit=1):
    """This walrus build accepts at most ONE sync-wait command per
    instruction. Move excess waits onto same-engine single-wait NoOps placed
    immediately before the instruction (identical blocking semantics)."""
    uid = 0
    for f in nc.m.functions:
        for bb in f.blocks:
            out = []
            for inst in bb.instructions:
                si = inst.sync_info
                if si is not None and si.on_wait and len(si.on_wait) > limit:
                    waits = list(si.on_wait)
                    excess, keep = waits[:-limit], waits[-limit:]
                    for k in range(0, len(excess), limit):
                        nop = mybir.InstNoOp(
                            name=f"wait_split_{uid}", ins=[], outs=[],
                            sync_info=mybir.SyncInfo(
                                on_wait=list(excess[k:k + limit]), on_update=[]))
                        nop.engine = inst.engine
                        nc.register_instruction(nop)
                        uid += 1
                        out.append(nop)
                    si.on_wait = keep
                out.append(inst)
            bb.instructions[:] = out
    return nc


def _build_program(split=True):
    nc = bass.Bass()
    f32 = mybir.dt.float32
    bf16 = mybir.dt.bfloat16
    # Host pre-arranges the shard partition-major so every tile load/store is
    # ONE <=3-dim DMA: xs[f, jhg, ci, bh, w], out[bt, dt, dh, co, bh, w].
    xs = nc.dram_tensor("xs", [TSH + 2, 2, C_IN, HB + 1, W + 2], bf16,
                        kind="ExternalInput")
    wt = nc.dram_tensor("wt", [6, 128, 128], bf16, kind="ExternalInput")
    bi = nc.dram_tensor("bi", [128, 1], f32, kind="ExternalInput")
    out = nc.dram_tensor("out", [NBT, 2, 2, C_OUT, HB, W], bf16,
                         kind="ExternalOutput")

    with tile.TileContext(nc) as tc:
        with tc.tile_pool(name="wpool", bufs=1) as wpool, \
             tc.tile_pool(name="xpool", bufs=2) as xpool, \
             tc.tile_pool(name="opool", bufs=3) as opool, \
             tc.tile_pool(name="pspool", bufs=2, space="PSUM") as pspool:
            w_sb = wpool.tile([128, 6, 128], bf16)
            nc.sync.dma_start(out=w_sb[:, :, :],
                              in_=wt.rearrange("i p m -> p i m"))
            b_sb = wpool.tile([128, 1], f32)
            nc.sync.dma_start(out=b_sb[:, :], in_=bi[:, :])

            # PE p-state warm-up: dummy matmuls on zeroed scratch while the
            # first input chunks stream in, so real matmuls start at full
            # clock. Reuses the regular ps{j} rotation (no extra PSUM).
            zw = wpool.tile([128, 128], bf16)
            zx = wpool.tile([128, 4, W + 2], bf16)
            nc.vector.memset(zw[:, :], 0)
            nc.vector.memset(zx[:, :, :], 0)
            for j in range(4):
                psw = pspool.tile([128, 4, W], f32, name=f"ps{j}")
                nc.tensor.matmul(psw[:, :, :], zw[:, :], zx[:, :, 0:W],
                                 start=True, stop=True)
            # ACT table preload off the critical path (first activation
            # pays ~1.3 us table load).
            zo = wpool.tile([128, 1], f32)
            nc.scalar.add(zo[:, :], zw[:, 0:1], add=0.0)

            # GPSIMD cannot read PSUM on trn2; split evictions DVE/ACT.
            ev_engine = [nc.vector, nc.scalar, nc.vector, nc.scalar]

            for bt in range(NBT):
                # 4 h-chunks, each GB+1=17 rows (1-row overlap at the top):
                # chunk g covers h-blocks [16g, 16g+16] so group g's matmuls
                # depend only on chunk g.
                x_g = [xpool.tile([128, GB + 1, W + 2], bf16, name=f"x{g}")
                       for g in range(NG)]
                src = xs[2 * bt:2 * bt + 4].rearrange("f j c b w -> (f j c) b w")
                for g in range(NG):
                    nc.sync.dma_start(
                        out=x_g[g][:, :, :],
                        in_=src[:, GB * g:GB * g + GB + 1, :])

                for g in range(NG):
                    out_g = opool.tile([128, GB, W], bf16, name="out_g")
                    for j in range(4):
                        ps = pspool.tile([128, 4, W], f32, name=f"ps{j}")
                        for i in range(6):
                            kw, jhh = divmod(i, 2)
                            rhs = x_g[g][:, 4 * j + jhh:4 * j + jhh + 4,
                                         kw:kw + W]
                            nc.tensor.matmul(ps[:, :, :], w_sb[:, i, :], rhs,
                                             start=(i == 0), stop=(i == 5))
                        eng = ev_engine[j]
                        if eng is nc.scalar:
                            eng.add(out_g[:, 4 * j:4 * j + 4, :],
                                    ps[:, :, :], add=b_sb[:, 0:1])
                        else:
                            eng.tensor_scalar_add(
                                out_g[:, 4 * j:4 * j + 4, :],
                                ps[:, :, :], b_sb[:, 0:1])
                    dst = out[bt].rearrange("dt dh co b w -> (dt dh co) b w")
                    nc.sync.dma_start(out=dst[:, GB * g:GB * (g + 1), :],
                                      in_=out_g[:, :, :])
    if split:
        _split_excess_waits(nc)
    return nc


_NC_CACHE = []


def _get_nc():
    if not _NC_CACHE:
        _NC_CACHE.append(_build_program())
    return _NC_CACHE[0]


def _pack_weights(weight):
    wt = np.zeros((6, 128, 128), np.float32)
    for kw in range(3):
        for jhh in range(2):
            i = kw * 2 + jhh
            for jt in range(4):
                for jhg in range(2):
                    jh = 2 * jhh + jhg
                    r0 = jt * 32 + jhg * 16
                    for dt in range(2):
                        kt = jt - dt
                        if not 0 <= kt < KS:
                            continue
                        for dh in range(2):
                            kh = jh - dh
                            if not 0 <= kh < KS:
                                continue
                            c0 = dt * 64 + dh * 32
                            wt[i, r0:r0 + 16, c0:c0 + 32] = \
                                weight[:, :, kt, kh, kw].T
    return wt.astype(ml_dtypes.bfloat16)


def run(x, weight, bias, trace=False):
    x = np.asarray(x, dtype=np.float32)
    weight = np.asarray(weight, dtype=np.float32)
    bias = np.asarray(bias, dtype=np.float32)

    xp = np.zeros((B, C_IN, T + 2, H + 2, W + 2), ml_dtypes.bfloat16)
    xp[:, :, 1:-1, 1:-1, 1:-1] = x.astype(ml_dtypes.bfloat16)
    wt = _pack_weights(weight)
    bi = np.tile(bias, 4).reshape(128, 1).astype(np.float32)

    in_maps = []
    for c in range(N_CORES):
        b, q = divmod(c, 4)
        t0 = q * TSH
        sh = xp[b, :, t0:t0 + TSH + 2]                # [ci, f, 130, 130]
        sh = sh.reshape(C_IN, TSH + 2, HB + 1, 2, W + 2)
        sh = np.ascontiguousarray(sh.transpose(1, 3, 0, 2, 4))
        in_maps.append({"xs": sh, "wt": wt, "bi": bi})

    nc = _get_nc()
    res = run_bass_kernel_spmd(nc, in_maps, list(range(N_CORES)), trace=trace)

    outp = np.empty((B, C_OUT, T, H, W), np.float32)
    for c in range(N_CORES):
        b, q = divmod(c, 4)
        r = res.results[c]["out"]                     # [bt, dt, dh, co, bh, w]
        r = np.asarray(r).astype(np.float32)
        r = r.transpose(3, 0, 1, 4, 2, 5).reshape(C_OUT, TSH, H, W)
        outp[b, :, q * TSH:(q + 1) * TSH] = r
    return outp, res


def kernel(x, weight, bias):
    outp, _ = run(x, weight, bias, trace=False)
    return outp


# revision 7
# speedup vs baseline: 1.3096x; 1.0142x over previous
"""Conv3d (k=3, pad=1) as shifted-window matmuls on 8 TRN2 NeuronCores.

Sharding: data-parallel over (batch B=2) x (T quarters of 8 output frames).
Each core computes out[b, :, t0:t0+8, :, :] from a host-padded input shard
xs[f, jhg, ci, bh, w] (conv zero-padding + t-halo baked in by the host).

Per-core formulation: output tile M=128 = (co=32, dt=2, dh=2) output
positions, contraction K=128 = (jt=4 t-window slots, jhg=2 h-parity, ci=16),
N=512 = (4 h-blocks x 128 w). The 3x3x3 kernel becomes 6 accumulating
matmuls (kw=3 x jhh=2) per PSUM tile, with all w/h shifts expressed as free-
dim AP offsets into one SBUF-resident tile.

v2: bf16 operands end-to-end on the device. bf16 weights enable the PE's
fast-weight-load path (fp32r LDWEIGHTS cost ~195 ns/matmul in the v1
trace); bf16 halves both input and output HBM traffic. PSUM accumulation
stays fp32; host casts the bf16 output back to fp32. Input loads are split
into 4 h-chunks per t-tile so the first matmuls start ~4x earlier, and
outputs are written per 16-row group to shorten the drain tail.
"""

import sys

if "/opt/trn_rl_repo" not in sys.path:
    sys.path.insert(0, "/opt/trn_rl_repo")

import numpy as np
import ml_dtypes

import concourse.bass as bass
import concourse.mybir as mybir
import concourse.tile as tile
from concourse.bass_utils import run_bass_kernel_spmd

B, C_IN, T, H, W = 2, 16, 32, 128, 128
C_OUT, KS = 32, 3
N_CORES = 8
TSH = T // 4          # output frames per core
NBT = TSH // 2        # bt tiles per core (2 output frames each)
HB = H // 2           # h blocks (dh=2)
NG = 4                # h-chunk groups per bt tile (16 h-blocks each)
GB = HB // NG         # h-blocks per group


def _split_excess_waits(nc, limit=1):
    """This walrus build accepts at most ONE sync-wait command per
    instruction. Move excess waits onto same-engine single-wait NoOps placed
    immediately before the instruction (identical blocking semantics)."""
    uid = 0
    for f in nc.m.functions:
        for bb in f.blocks:
            out = []
            for inst in bb.instructions:
                si = inst.sync_info
                if si is not None and si.on_wait and len(si.on_wait) > limit:
                    waits = list(si.on_wait)
                    excess, keep = waits[:-limit], waits[-limit:]
                    for k in range(0, len(excess), limit):
                        nop = mybir.InstNoOp(
                            name=f"wait_split_{uid}", ins=[], outs=[],
                            sync_info=mybir.SyncInfo(
                                on_wait=list(excess[k:k + limit]), on_update=[]))
                        nop.engine = inst.engine
                        nc.register_instruction(nop)
                        uid += 1
                        out.append(nop)
                    si.on_wait = keep
                out.append(inst)
            bb.instructions[:] = out
    return nc


def _build_program(split=True):
    nc = bass.Bass()
    f32 = mybir.dt.float32
    bf16 = mybir.dt.bfloat16
    # Host pre-arranges the shard partition-major so every tile load/store is
    # ONE <=3-dim DMA: xs[f, jhg, ci, bh, w], out[bt, dt, dh, co, bh, w].
    xs = nc.dram_tensor("xs", [TSH + 2, 2, C_IN, HB + 1, W + 2], bf16,
                        kind="ExternalInput")
    wt = nc.dram_tensor("wt", [6, 128, 128], bf16, kind="ExternalInput")
    bi = nc.dram_tensor("bi", [128, 1], f32, kind="ExternalInput")
    out = nc.dram_tensor("out", [NBT, 2, 2, C_OUT, HB, W], bf16,
                         kind="ExternalOutput")

    with tile.TileContext(nc) as tc:
        with tc.tile_pool(name="wpool", bufs=1) as wpool, \
             tc.tile_pool(name="xpool", bufs=3) as xpool, \
             tc.tile_pool(name="opool", bufs=3) as opool, \
             tc.tile_pool(name="pspool", bufs=2, space="PSUM") as pspool:
            w_sb = wpool.tile([128, 6, 128], bf16)
            nc.sync.dma_start(out=w_sb[:, :, :],
                              in_=wt.rearrange("i p m -> p i m"))
            b_sb = wpool.tile([128, 1], f32)
            nc.sync.dma_start(out=b_sb[:, :], in_=bi[:, :])

            # PE p-state warm-up: dummy matmuls on zeroed scratch while the
            # first input chunks stream in, so real matmuls start at full
            # clock. Reuses the regular ps{j} rotation (no extra PSUM).
            zw = wpool.tile([128, 128], bf16)
            zx = wpool.tile([128, 4, W + 2], bf16)
            nc.vector.memset(zw[:, :], 0)
            nc.vector.memset(zx[:, :, :], 0)
            for j in range(4):
                psw = pspool.tile([128, 4, W], f32, name=f"ps{j}")
                nc.tensor.matmul(psw[:, :, :], zw[:, :], zx[:, :, 0:W],
                                 start=True, stop=True)
            # ACT table preload off the critical path (first activation
            # pays ~1.3 us table load).
            zo = wpool.tile([128, 1], f32)
            nc.scalar.add(zo[:, :], zw[:, 0:1], add=0.0)

            # GPSIMD cannot read PSUM on trn2; split evictions DVE/ACT.
            ev_engine = [nc.vector, nc.scalar, nc.vector, nc.scalar]

            for bt in range(NBT):
                # 4 h-chunks, each GB+1=17 rows (1-row overlap at the top):
                # chunk g covers h-blocks [16g, 16g+16] so group g's matmuls
                # depend only on chunk g.
                x_g = [xpool.tile([128, GB + 1, W + 2], bf16, name=f"x{g}")
                       for g in range(NG)]
                src = xs[2 * bt:2 * bt + 4].rearrange("f j c b w -> (f j c) b w")
                for g in range(NG):
                    # x loads issue from the otherwise-idle GPSIMD queue so
                    # they are not serialized behind Sync's preamble/out-DMAs.
                    nc.gpsimd.dma_start(
                        out=x_g[g][:, :, :],
                        in_=src[:, GB * g:GB * g + GB + 1, :])

                for g in range(NG):
                    out_g = opool.tile([128, GB, W], bf16, name="out_g")
                    for j in range(4):
                        ps = pspool.tile([128, 4, W], f32, name=f"ps{j}")
                        for i in range(6):
                            kw, jhh = divmod(i, 2)
                            rhs = x_g[g][:, 4 * j + jhh:4 * j + jhh + 4,
                                         kw:kw + W]
                            nc.tensor.matmul(ps[:, :, :], w_sb[:, i, :], rhs,
                                             start=(i == 0), stop=(i == 5))
                        eng = ev_engine[j]
                        if eng is nc.scalar:
                            eng.add(out_g[:, 4 * j:4 * j + 4, :],
                                    ps[:, :, :], add=b_sb[:, 0:1])
                        else:
                            eng.tensor_scalar_add(
                                out_g[:, 4 * j:4 * j + 4, :],
                                ps[:, :, :], b_sb[:, 0:1])
                    dst = out[bt].rearrange("dt dh co b w -> (dt dh co) b w")
                    nc.sync.dma_start(out=dst[:, GB * g:GB * (g + 1), :],
                                      in_=out_g[:, :, :])
    if split:
        _split_excess_waits(nc)
    return nc


_NC_CACHE = []


def _get_nc():
    if not _NC_CACHE:
        _NC_CACHE.append(_build_program())
    return _NC_CACHE[0]


def _pack_weights(weight):
    wt = np.zeros((6, 128, 128), np.float32)
    for kw in range(3):
        for jhh in range(2):
            i = kw * 2 + jhh
            for jt in range(4):
                for jhg in range(2):
                    jh = 2 * jhh + jhg
                    r0 = jt * 32 + jhg * 16
                    for dt in range(2):
                        kt = jt - dt
                        if not 0 <= kt < KS:
                            continue
                        for dh in range(2):
                            kh = jh - dh
                            if not 0 <= kh < KS:
                                continue
                            c0 = dt * 64 + dh * 32
                            wt[i, r0:r0 + 16, c0:c0 + 32] = \
                                weight[:, :, kt, kh, kw].T
    return wt.astype(ml_dtypes.bfloat16)


def run(x, weight, bias, trace=False):
    x = np.asarray(x, dtype=np.float32)
    weight = np.asarray(weight, dtype=np.float32)
    bias = np.asarray(bias, dtype=np.float32)

    xp = np.zeros((B, C_IN, T + 2, H + 2, W + 2), ml_dtypes.bfloat16)
    xp[:, :, 1:-1, 1:-1, 1:-1] = x.astype(ml_dtypes.bfloat16)
    wt = _pack_weights(weight)
    bi = np.tile(bias, 4).reshape(128, 1).astype(np.float32)

    in_maps = []
    for c in range(N_CORES):
        b, q = divmod(c, 4)
        t0 = q * TSH
        sh = xp[b, :, t0:t0 + TSH + 2]                # [ci, f, 130, 130]
        sh = sh.reshape(C_IN, TSH + 2, HB + 1, 2, W + 2)
        sh = np.ascontiguousarray(sh.transpose(1, 3, 0, 2, 4))
        in_maps.append({"xs": sh, "wt": wt, "bi": bi})

    nc = _get_nc()
    res = run_bass_kernel_spmd(nc, in_maps, list(range(N_CORES)), trace=trace)

    outp = np.empty((B, C_OUT, T, H, W), np.float32)
    for c in range(N_CORES):
        b, q = divmod(c, 4)
        r = res.results[c]["out"]                     # [bt, dt, dh, co, bh, w]
        r = np.asarray(r).astype(np.float32)
        r = r.transpose(3, 0, 1, 4, 2, 5).reshape(C_OUT, TSH, H, W)
        outp[b, :, q * TSH:(q + 1) * TSH] = r
    return outp, res


def kernel(x, weight, bias):
    outp, _ = run(x, weight, bias, trace=False)
    return outp


# revision 11
# speedup vs baseline: 1.3217x; 1.0092x over previous
"""Conv3d (k=3, pad=1) as shifted-window matmuls on 8 TRN2 NeuronCores.

Sharding: data-parallel over (batch B=2) x (T quarters of 8 output frames).
Each core computes out[b, :, t0:t0+8, :, :] from a host-padded input shard
xs[f, jhg, ci, bh, w] (conv zero-padding + t-halo baked in by the host).

Per-core formulation: output tile M=128 = (co=32, dt=2, dh=2) output
positions, contraction K=128 = (jt=4 t-window slots, jhg=2 h-parity, ci=16),
N=512 = (4 h-blocks x 128 w). The 3x3x3 kernel becomes 6 accumulating
matmuls (kw=3 x jhh=2) per PSUM tile, with all w/h shifts expressed as free-
dim AP offsets into one SBUF-resident tile.

v2: bf16 operands end-to-end on the device. bf16 weights enable the PE's
fast-weight-load path (fp32r LDWEIGHTS cost ~195 ns/matmul in the v1
trace); bf16 halves both input and output HBM traffic. PSUM accumulation
stays fp32; host casts the bf16 output back to fp32. Input loads are split
into 4 h-chunks per t-tile so the first matmuls start ~4x earlier, and
outputs are written per 16-row group to shorten the drain tail.
"""

import sys

if "/opt/trn_rl_repo" not in sys.path:
    sys.path.insert(0, "/opt/trn_rl_repo")

import numpy as np
import ml_dtypes

import concourse.bass as bass
import concourse.mybir as mybir
import concourse.tile as tile
from concourse.bass_utils import run_bass_kernel_spmd

B, C_IN, T, H, W = 2, 16, 32, 128, 128
C_OUT, KS = 32, 3
N_CORES = 8
TSH = T // 4          # output frames per core
NBT = TSH // 2        # bt tiles per core (2 output frames each)
HB = H // 2           # h blocks (dh=2)
NG = 4                # h-chunk groups per bt tile (16 h-blocks each)
GB = HB // NG         # h-blocks per group


def _split_excess_waits(nc, limit=1):
    """This walrus build accepts at most ONE sync-wait command per
    instruction. Move excess waits onto same-engine single-wait NoOps placed
    immediately before the instruction (identical blocking semantics)."""
    uid = 0
    for f in nc.m.functions:
        for bb in f.blocks:
            out = []
            for inst in bb.instructions:
                si = inst.sync_info
                if si is not None and si.on_wait and len(si.on_wait) > limit:
                    waits = list(si.on_wait)
                    excess, keep = waits[:-limit], waits[-limit:]
                    for k in range(0, len(excess), limit):
                        nop = mybir.InstNoOp(
                            name=f"wait_split_{uid}", ins=[], outs=[],
                            sync_info=mybir.SyncInfo(
                                on_wait=list(excess[k:k + limit]), on_update=[]))
                        nop.engine = inst.engine
                        nc.register_instruction(nop)
                        uid += 1
                        out.append(nop)
                    si.on_wait = keep
                out.append(inst)
            bb.instructions[:] = out
    return nc


def _build_program(split=True):
    nc = bass.Bass()
    f32 = mybir.dt.float32
    bf16 = mybir.dt.bfloat16
    # Host pre-arranges the shard partition-major so every tile load/store is
    # ONE <=3-dim DMA: xs[f, jhg, ci, bh, w], out[bt, dt, dh, co, bh, w].
    xs = nc.dram_tensor("xs", [TSH + 2, 2, C_IN, HB + 1, W + 2], bf16,
                        kind="ExternalInput")
    wt = nc.dram_tensor("wt", [6, 128, 128], bf16, kind="ExternalInput")
    bi = nc.dram_tensor("bi", [128, 1], f32, kind="ExternalInput")
    out = nc.dram_tensor("out", [NBT, 2, 2, C_OUT, HB, W], bf16,
                         kind="ExternalOutput")

    with tile.TileContext(nc) as tc:
        with tc.tile_pool(name="wpool", bufs=1) as wpool, \
             tc.tile_pool(name="xpool", bufs=3) as xpool, \
             tc.tile_pool(name="opool", bufs=3) as opool, \
             tc.tile_pool(name="pspool", bufs=2, space="PSUM") as pspool:
            w_sb = wpool.tile([128, 6, 128], bf16)
            # Weights from the GPSIMD queue, first x chunk from Sync: both
            # issue right at the post-preamble barrier (~7.3 us) in parallel.
            nc.gpsimd.dma_start(out=w_sb[:, :, :],
                                in_=wt.rearrange("i p m -> p i m"))
            b_sb = wpool.tile([128, 1], f32)
            nc.gpsimd.dma_start(out=b_sb[:, :], in_=bi[:, :])

            # PE p-state warm-up: dummy matmuls on zeroed scratch while the
            # first input chunks stream in, so real matmuls start at full
            # clock. Reuses the regular ps{j} rotation (no extra PSUM).
            zw = wpool.tile([128, 128], bf16)
            zx = wpool.tile([128, 4, W + 2], bf16)
            nc.vector.memset(zw[:, :], 0)
            nc.vector.memset(zx[:, :, :], 0)
            for j in range(6):
                psw = pspool.tile([128, 4, W], f32, name=f"ps{j % 4}")
                nc.tensor.matmul(psw[:, :, :], zw[:, :], zx[:, :, 0:W],
                                 start=True, stop=True)
            # ACT table preload off the critical path (first activation
            # pays ~1.3 us table load).
            zo = wpool.tile([128, 1], f32)
            nc.scalar.add(zo[:, :], zw[:, 0:1], add=0.0)

            # GPSIMD cannot read PSUM on trn2; split evictions DVE/ACT.
            ev_engine = [nc.vector, nc.scalar, nc.vector, nc.scalar]

            for bt in range(NBT):
                # 4 h-chunks, each GB+1=17 rows (1-row overlap at the top):
                # chunk g covers h-blocks [16g, 16g+16] so group g's matmuls
                # depend only on chunk g.
                x_g = [xpool.tile([128, GB + 1, W + 2], bf16, name=f"x{g}")
                       for g in range(NG)]
                src = xs[2 * bt:2 * bt + 4].rearrange("f j c b w -> (f j c) b w")
                for g in range(NG):
                    # x loads issue from the otherwise-idle GPSIMD queue so
                    # they are not serialized behind Sync's preamble/out-DMAs
                    # — except the very first chunk, which goes on Sync to
                    # start in parallel with the GPSIMD-issued weight load.
                    eng = nc.sync if (bt == 0 and g == 0) else nc.gpsimd
                    eng.dma_start(
                        out=x_g[g][:, :, :],
                        in_=src[:, GB * g:GB * g + GB + 1, :])

                for g in range(NG):
                    out_g = opool.tile([128, GB, W], bf16, name="out_g")
                    for j in range(4):
                        ps = pspool.tile([128, 4, W], f32, name=f"ps{j}")
                        for i in range(6):
                            kw, jhh = divmod(i, 2)
                            rhs = x_g[g][:, 4 * j + jhh:4 * j + jhh + 4,
                                         kw:kw + W]
                            nc.tensor.matmul(ps[:, :, :], w_sb[:, i, :], rhs,
                                             start=(i == 0), stop=(i == 5))
                        eng = ev_engine[j]
                        if eng is nc.scalar:
                            eng.add(out_g[:, 4 * j:4 * j + 4, :],
                                    ps[:, :, :], add=b_sb[:, 0:1])
                        else:
                            eng.tensor_scalar_add(
                                out_g[:, 4 * j:4 * j + 4, :],
                                ps[:, :, :], b_sb[:, 0:1])
                        if j % 2 == 1:
                            # write each 8-row half as soon as its two
                            # evictions land — halves the drain tail.
                            dst = out[bt].rearrange(
                                "dt dh co b w -> (dt dh co) b w")
                            h0 = GB * g + 4 * (j - 1)
                            nc.sync.dma_start(
                                out=dst[:, h0:h0 + 8, :],
                                in_=out_g[:, 4 * (j - 1):4 * (j + 1), :])
    if split:
        _split_excess_waits(nc)
    return nc


_NC_CACHE = []


def _get_nc():
    if not _NC_CACHE:
        _NC_CACHE.append(_build_program())
    return _NC_CACHE[0]


def _pack_weights(weight):
    wt = np.zeros((6, 128, 128), np.float32)
    for kw in range(3):
        for jhh in range(2):
            i = kw * 2 + jhh
            for jt in range(4):
                for jhg in range(2):
                    jh = 2 * jhh + jhg
                    r0 = jt * 32 + jhg * 16
                    for dt in range(2):
                        kt = jt - dt
                        if not 0 <= kt < KS:
                            continue
                        for dh in range(2):
                            kh = jh - dh
                            if not 0 <= kh < KS:
                                continue
                            c0 = dt * 64 + dh * 32
                            wt[i, r0:r0 + 16, c0:c0 + 32] = \
                                weight[:, :, kt, kh, kw].T
    return wt.astype(ml_dtypes.bfloat16)


def run(x, weight, bias, trace=False):
    x = np.asarray(x, dtype=np.float32)
    weight = np.asarray(weight, dtype=np.float32)
    bias = np.asarray(bias, dtype=np.float32)

    xp = np.zeros((B, C_IN, T + 2, H + 2, W + 2), ml_dtypes.bfloat16)
    xp[:, :, 1:-1, 1:-1, 1:-1] = x.astype(ml_dtypes.bfloat16)
    wt = _pack_weights(weight)
    bi = np.tile(bias, 4).reshape(128, 1).astype(np.float32)

    in_maps = []
    for c in range(N_CORES):
        b, q = divmod(c, 4)
        t0 = q * TSH
        sh = xp[b, :, t0:t0 + TSH + 2]                # [ci, f, 130, 130]
        sh = sh.reshape(C_IN, TSH + 2, HB + 1, 2, W + 2)
        sh = np.ascontiguousarray(sh.transpose(1, 3, 0, 2, 4))
        in_maps.append({"xs": sh, "wt": wt, "bi": bi})

    nc = _get_nc()
    res = run_bass_kernel_spmd(nc, in_maps, list(range(N_CORES)), trace=trace)

    outp = np.empty((B, C_OUT, T, H, W), np.float32)
    for c in range(N_CORES):
        b, q = divmod(c, 4)
        r = res.results[c]["out"]                     # [bt, dt, dh, co, bh, w]
        r = np.asarray(r).astype(np.float32)
        r = r.transpose(3, 0, 1, 4, 2, 5).reshape(C_OUT, TSH, H, W)
        outp[b, :, q * TSH:(q + 1) * TSH] = r
    return outp, res


def kernel(x, weight, bias):
    outp, _ = run(x, weight, bias, trace=False)
    return outp


# revision 15
# speedup vs baseline: 1.3343x; 1.0095x over previous
"""Conv3d (k=3, pad=1) as shifted-window matmuls on 8 TRN2 NeuronCores.

Sharding: data-parallel over (batch B=2) x (T quarters of 8 output frames).
Each core computes out[b, :, t0:t0+8, :, :] from a host-padded input shard
xs[f, jhg, ci, bh, w] (conv zero-padding + t-halo baked in by the host).

Per-core formulation: output tile M=128 = (co=32, dt=2, dh=2) output
positions, contraction K=128 = (jt=4 t-window slots, jhg=2 h-parity, ci=16),
N=512 = (4 h-blocks x 128 w). The 3x3x3 kernel becomes 6 accumulating
matmuls (kw=3 x jhh=2) per PSUM tile, with all w/h shifts expressed as free-
dim AP offsets into one SBUF-resident tile.

v2: bf16 operands end-to-end on the device. bf16 weights enable the PE's
fast-weight-load path (fp32r LDWEIGHTS cost ~195 ns/matmul in the v1
trace); bf16 halves both input and output HBM traffic. PSUM accumulation
stays fp32; host casts the bf16 output back to fp32. Input loads are split
into 4 h-chunks per t-tile so the first matmuls start ~4x earlier, and
outputs are written per 16-row group to shorten the drain tail.
"""

import sys

if "/opt/trn_rl_repo" not in sys.path:
    sys.path.insert(0, "/opt/trn_rl_repo")

import numpy as np
import ml_dtypes

import concourse.bass as bass
import concourse.mybir as mybir
import concourse.tile as tile
from concourse.bass_utils import run_bass_kernel_spmd

B, C_IN, T, H, W = 2, 16, 32, 128, 128
C_OUT, KS = 32, 3
N_CORES = 8
TSH = T // 4          # output frames per core
NBT = TSH // 2        # bt tiles per core (2 output frames each)
HB = H // 2           # h blocks (dh=2)
NG = 4                # h-chunk groups per bt tile (16 h-blocks each)
GB = HB // NG         # h-blocks per group


def _split_excess_waits(nc, limit=1):
    """This walrus build accepts at most ONE sync-wait command per
    instruction. Move excess waits onto same-engine single-wait NoOps placed
    immediately before the instruction (identical blocking semantics)."""
    uid = 0
    for f in nc.m.functions:
        for bb in f.blocks:
            out = []
            for inst in bb.instructions:
                si = inst.sync_info
                if si is not None and si.on_wait and len(si.on_wait) > limit:
                    waits = list(si.on_wait)
                    excess, keep = waits[:-limit], waits[-limit:]
                    for k in range(0, len(excess), limit):
                        nop = mybir.InstNoOp(
                            name=f"wait_split_{uid}", ins=[], outs=[],
                            sync_info=mybir.SyncInfo(
                                on_wait=list(excess[k:k + limit]), on_update=[]))
                        nop.engine = inst.engine
                        nc.register_instruction(nop)
                        uid += 1
                        out.append(nop)
                    si.on_wait = keep
                out.append(inst)
            bb.instructions[:] = out
    return nc


def _build_program(split=True):
    nc = bass.Bass()
    f32 = mybir.dt.float32
    bf16 = mybir.dt.bfloat16
    # Host pre-arranges the shard partition-major so every tile load/store is
    # ONE <=3-dim DMA: xs[f, jhg, ci, bh, w], out[bt, dt, dh, co, bh, w].
    xs = nc.dram_tensor("xs", [TSH + 2, 2, C_IN, HB + 1, W + 2], bf16,
                        kind="ExternalInput")
    wt = nc.dram_tensor("wt", [128, 6, 128], bf16, kind="ExternalInput")
    bi = nc.dram_tensor("bi", [128, 1], f32, kind="ExternalInput")
    out = nc.dram_tensor("out", [NBT, 2, 2, C_OUT, HB, W], bf16,
                         kind="ExternalOutput")

    with tile.TileContext(nc) as tc:
        with tc.tile_pool(name="wpool", bufs=1) as wpool, \
             tc.tile_pool(name="xpool", bufs=3) as xpool, \
             tc.tile_pool(name="opool", bufs=3) as opool, \
             tc.tile_pool(name="pspool", bufs=2, space="PSUM") as pspool:
            w_sb = wpool.tile([128, 6, 128], bf16)
            # Weights from the GPSIMD queue, first x chunk from Sync: both
            # issue right at the post-preamble barrier (~7.3 us) in parallel.
            # wt is host-packed partition-major so this is 128 contiguous
            # 1.5 KB packets instead of 768 x 256 B.
            nc.gpsimd.dma_start(out=w_sb[:, :, :], in_=wt[:, :, :])
            b_sb = wpool.tile([128, 1], f32)
            nc.gpsimd.dma_start(out=b_sb[:, :], in_=bi[:, :])

            # PE p-state warm-up: dummy matmuls on zeroed scratch while the
            # first input chunks stream in, so real matmuls start at full
            # clock. Reuses the regular ps{j} rotation (no extra PSUM).
            zw = wpool.tile([128, 128], bf16)
            zx = wpool.tile([128, 4, W + 2], bf16)
            nc.vector.memset(zw[:, :], 0)
            nc.vector.memset(zx[:, :, :], 0)
            for j in range(7):
                psw = pspool.tile([128, 4, W], f32, name=f"ps{j % 4}")
                nc.tensor.matmul(psw[:, :, :], zw[:, :], zx[:, :, 0:W],
                                 start=True, stop=True)
            # ACT table preload off the critical path (first activation
            # pays ~1.3 us table load).
            zo = wpool.tile([128, 1], f32)
            nc.scalar.add(zo[:, :], zw[:, 0:1], add=0.0)

            # GPSIMD cannot read PSUM on trn2; split evictions DVE/ACT.
            ev_engine = [nc.vector, nc.scalar, nc.vector, nc.scalar]

            for bt in range(NBT):
                # 4 h-chunks, each GB+1=17 rows (1-row overlap at the top):
                # chunk g covers h-blocks [16g, 16g+16] so group g's matmuls
                # depend only on chunk g.
                x_g = [xpool.tile([128, GB + 1, W + 2], bf16, name=f"x{g}")
                       for g in range(NG)]
                src = xs[2 * bt:2 * bt + 4].rearrange("f j c b w -> (f j c) b w")
                for g in range(NG):
                    # x loads issue from the otherwise-idle GPSIMD queue so
                    # they are not serialized behind Sync's preamble/out-DMAs
                    # — except the very first chunk, which goes on Sync to
                    # start in parallel with the GPSIMD-issued weight load.
                    eng = nc.sync if (bt == 0 and g == 0) else nc.gpsimd
                    eng.dma_start(
                        out=x_g[g][:, :, :],
                        in_=src[:, GB * g:GB * g + GB + 1, :])

                for g in range(NG):
                    out_g = opool.tile([128, GB, W], bf16, name="out_g")
                    for j in range(4):
                        ps = pspool.tile([128, 4, W], f32, name=f"ps{j}")
                        for i in range(6):
                            kw, jhh = divmod(i, 2)
                            rhs = x_g[g][:, 4 * j + jhh:4 * j + jhh + 4,
                                         kw:kw + W]
                            nc.tensor.matmul(ps[:, :, :], w_sb[:, i, :], rhs,
                                             start=(i == 0), stop=(i == 5))
                        eng = ev_engine[j]
                        if eng is nc.scalar:
                            eng.add(out_g[:, 4 * j:4 * j + 4, :],
                                    ps[:, :, :], add=b_sb[:, 0:1])
                        else:
                            eng.tensor_scalar_add(
                                out_g[:, 4 * j:4 * j + 4, :],
                                ps[:, :, :], b_sb[:, 0:1])
                        if j % 2 == 1:
                            # write each 8-row half as soon as its two
                            # evictions land — halves the drain tail.
                            dst = out[bt].rearrange(
                                "dt dh co b w -> (dt dh co) b w")
                            h0 = GB * g + 4 * (j - 1)
                            nc.sync.dma_start(
                                out=dst[:, h0:h0 + 8, :],
                                in_=out_g[:, 4 * (j - 1):4 * (j + 1), :])
    if split:
        _split_excess_waits(nc)
    return nc


_NC_CACHE = []


def _get_nc():
    if not _NC_CACHE:
        _NC_CACHE.append(_build_program())
    return _NC_CACHE[0]


def _pack_weights(weight):
    wt = np.zeros((6, 128, 128), np.float32)
    for kw in range(3):
        for jhh in range(2):
            i = kw * 2 + jhh
            for jt in range(4):
                for jhg in range(2):
                    jh = 2 * jhh + jhg
                    r0 = jt * 32 + jhg * 16
                    for dt in range(2):
                        kt = jt - dt
                        if not 0 <= kt < KS:
                            continue
                        for dh in range(2):
                            kh = jh - dh
                            if not 0 <= kh < KS:
                                continue
                            c0 = dt * 64 + dh * 32
                            wt[i, r0:r0 + 16, c0:c0 + 32] = \
                                weight[:, :, kt, kh, kw].T
    # partition-major [p, i, m] so the SBUF load is one big-packet DMA
    return np.ascontiguousarray(
        wt.transpose(1, 0, 2)).astype(ml_dtypes.bfloat16)


def run(x, weight, bias, trace=False):
    x = np.asarray(x, dtype=np.float32)
    weight = np.asarray(weight, dtype=np.float32)
    bias = np.asarray(bias, dtype=np.float32)

    xp = np.zeros((B, C_IN, T + 2, H + 2, W + 2), ml_dtypes.bfloat16)
    xp[:, :, 1:-1, 1:-1, 1:-1] = x.astype(ml_dtypes.bfloat16)
    wt = _pack_weights(weight)
    bi = np.tile(bias, 4).reshape(128, 1).astype(np.float32)

    in_maps = []
    for c in range(N_CORES):
        b, q = divmod(c, 4)
        t0 = q * TSH
        sh = xp[b, :, t0:t0 + TSH + 2]                # [ci, f, 130, 130]
        sh = sh.reshape(C_IN, TSH + 2, HB + 1, 2, W + 2)
        sh = np.ascontiguousarray(sh.transpose(1, 3, 0, 2, 4))
        in_maps.append({"xs": sh, "wt": wt, "bi": bi})

    nc = _get_nc()
    res = run_bass_kernel_spmd(nc, in_maps, list(range(N_CORES)), trace=trace)

    outp = np.empty((B, C_OUT, T, H, W), np.float32)
    for c in range(N_CORES):
        b, q = divmod(c, 4)
        r = res.results[c]["out"]                     # [bt, dt, dh, co, bh, w]
        r = np.asarray(r).astype(np.float32)
        r = r.transpose(3, 0, 1, 4, 2, 5).reshape(C_OUT, TSH, H, W)
        outp[b, :, q * TSH:(q + 1) * TSH] = r
    return outp, res


def kernel(x, weight, bias):
    outp, _ = run(x, weight, bias, trace=False)
    return outp


# revision 17
# speedup vs baseline: 1.3386x; 1.0032x over previous
"""Conv3d (k=3, pad=1) as shifted-window matmuls on 8 TRN2 NeuronCores.

Sharding: data-parallel over (batch B=2) x (T quarters of 8 output frames).
Each core computes out[b, :, t0:t0+8, :, :] from a host-padded input shard
xs[f, jhg, ci, bh, w] (conv zero-padding + t-halo baked in by the host).

Per-core formulation: output tile M=128 = (co=32, dt=2, dh=2) output
positions, contraction K=128 = (jt=4 t-window slots, jhg=2 h-parity, ci=16),
N=512 = (4 h-blocks x 128 w). The 3x3x3 kernel becomes 6 accumulating
matmuls (kw=3 x jhh=2) per PSUM tile, with all w/h shifts expressed as free-
dim AP offsets into one SBUF-resident tile.

v2: bf16 operands end-to-end on the device. bf16 weights enable the PE's
fast-weight-load path (fp32r LDWEIGHTS cost ~195 ns/matmul in the v1
trace); bf16 halves both input and output HBM traffic. PSUM accumulation
stays fp32; host casts the bf16 output back to fp32. Input loads are split
into 4 h-chunks per t-tile so the first matmuls start ~4x earlier, and
outputs are written per 16-row group to shorten the drain tail.
"""

import sys

if "/opt/trn_rl_repo" not in sys.path:
    sys.path.insert(0, "/opt/trn_rl_repo")

import numpy as np
import ml_dtypes

import concourse.bass as bass
import concourse.mybir as mybir
import concourse.tile as tile
from concourse.bass_utils import run_bass_kernel_spmd

B, C_IN, T, H, W = 2, 16, 32, 128, 128
C_OUT, KS = 32, 3
N_CORES = 8
TSH = T // 4          # output frames per core
NBT = TSH // 2        # bt tiles per core (2 output frames each)
HB = H // 2           # h blocks (dh=2)
NG = 4                # h-chunk groups per bt tile (16 h-blocks each)
GB = HB // NG         # h-blocks per group


def _split_excess_waits(nc, limit=1):
    """This walrus build accepts at most ONE sync-wait command per
    instruction. Move excess waits onto same-engine single-wait NoOps placed
    immediately before the instruction (identical blocking semantics)."""
    uid = 0
    for f in nc.m.functions:
        for bb in f.blocks:
            out = []
            for inst in bb.instructions:
                si = inst.sync_info
                if si is not None and si.on_wait and len(si.on_wait) > limit:
                    waits = list(si.on_wait)
                    excess, keep = waits[:-limit], waits[-limit:]
                    for k in range(0, len(excess), limit):
                        nop = mybir.InstNoOp(
                            name=f"wait_split_{uid}", ins=[], outs=[],
                            sync_info=mybir.SyncInfo(
                                on_wait=list(excess[k:k + limit]), on_update=[]))
                        nop.engine = inst.engine
                        nc.register_instruction(nop)
                        uid += 1
                        out.append(nop)
                    si.on_wait = keep
                out.append(inst)
            bb.instructions[:] = out
    return nc


def _build_program(split=True):
    nc = bass.Bass()
    f32 = mybir.dt.float32
    bf16 = mybir.dt.bfloat16
    # Host pre-arranges the shard partition-major so every tile load/store is
    # ONE <=3-dim DMA: xs[f, jhg, ci, bh, w], out[bt, dt, dh, co, bh, w].
    xs = nc.dram_tensor("xs", [TSH + 2, 2, C_IN, HB + 1, W + 2], bf16,
                        kind="ExternalInput")
    wt = nc.dram_tensor("wt", [128, 6, 128], bf16, kind="ExternalInput")
    bi = nc.dram_tensor("bi", [128, 1], f32, kind="ExternalInput")
    out = nc.dram_tensor("out", [NBT, 2, 2, C_OUT, HB, W], bf16,
                         kind="ExternalOutput")

    with tile.TileContext(nc) as tc:
        with tc.tile_pool(name="wpool", bufs=1) as wpool, \
             tc.tile_pool(name="xpool", bufs=3) as xpool, \
             tc.tile_pool(name="opool", bufs=3) as opool, \
             tc.tile_pool(name="pspool", bufs=2, space="PSUM") as pspool:
            w_sb = wpool.tile([128, 6, 128], bf16)
            # Weights from the GPSIMD queue, first x chunk from Sync: both
            # issue right at the post-preamble barrier (~7.3 us) in parallel.
            # wt is host-packed partition-major so this is 128 contiguous
            # 1.5 KB packets instead of 768 x 256 B.
            nc.gpsimd.dma_start(out=w_sb[:, :, :], in_=wt[:, :, :])
            b_sb = wpool.tile([128, 1], f32)
            nc.gpsimd.dma_start(out=b_sb[:, :], in_=bi[:, :])

            # PE p-state warm-up: dummy matmuls on zeroed scratch while the
            # first input chunks stream in, so real matmuls start at full
            # clock. Reuses the regular ps{j} rotation (no extra PSUM).
            zw = wpool.tile([128, 128], bf16)
            zx = wpool.tile([128, 4, W + 2], bf16)
            nc.vector.memset(zw[:, :], 0)
            nc.vector.memset(zx[:, :, :], 0)
            for j in range(7):
                psw = pspool.tile([128, 4, W], f32, name=f"ps{j % 4}")
                nc.tensor.matmul(psw[:, :, :], zw[:, :], zx[:, :, 0:W],
                                 start=True, stop=True)
            # ACT table preload off the critical path (first activation
            # pays ~1.3 us table load).
            zo = wpool.tile([128, 1], f32)
            nc.scalar.add(zo[:, :], zw[:, 0:1], add=0.0)

            # GPSIMD cannot read PSUM on trn2; split evictions DVE/ACT.
            ev_engine = [nc.vector, nc.scalar, nc.vector, nc.scalar]

            for bt in range(NBT):
                # 4 h-chunks, each GB+1=17 rows (1-row overlap at the top):
                # chunk g covers h-blocks [16g, 16g+16] so group g's matmuls
                # depend only on chunk g.
                x_g = [xpool.tile([128, GB + 1, W + 2], bf16, name=f"x{g}")
                       for g in range(NG)]
                src = xs[2 * bt:2 * bt + 4].rearrange("f j c b w -> (f j c) b w")
                for g in range(NG):
                    # x loads issue from the otherwise-idle GPSIMD queue so
                    # they are not serialized behind Sync's preamble/out-DMAs.
                    # The very first chunk gates the whole stream start, so
                    # it is split into 3 disjoint row-ranges landing in
                    # parallel on Sync (2) + GPSIMD (1).
                    if bt == 0 and g == 0:
                        nc.sync.dma_start(out=x_g[0][:, 0:6, :],
                                          in_=src[:, 0:6, :])
                        nc.sync.dma_start(out=x_g[0][:, 6:10, :],
                                          in_=src[:, 6:10, :])
                        nc.gpsimd.dma_start(out=x_g[0][:, 10:17, :],
                                            in_=src[:, 10:17, :])
                    else:
                        nc.gpsimd.dma_start(
                            out=x_g[g][:, :, :],
                            in_=src[:, GB * g:GB * g + GB + 1, :])

                for g in range(NG):
                    out_g = opool.tile([128, GB, W], bf16, name="out_g")
                    for j in range(4):
                        ps = pspool.tile([128, 4, W], f32, name=f"ps{j}")
                        for i in range(6):
                            kw, jhh = divmod(i, 2)
                            rhs = x_g[g][:, 4 * j + jhh:4 * j + jhh + 4,
                                         kw:kw + W]
                            nc.tensor.matmul(ps[:, :, :], w_sb[:, i, :], rhs,
                                             start=(i == 0), stop=(i == 5))
                        eng = ev_engine[j]
                        if eng is nc.scalar:
                            eng.add(out_g[:, 4 * j:4 * j + 4, :],
                                    ps[:, :, :], add=b_sb[:, 0:1])
                        else:
                            eng.tensor_scalar_add(
                                out_g[:, 4 * j:4 * j + 4, :],
                                ps[:, :, :], b_sb[:, 0:1])
                        dst = out[bt].rearrange(
                            "dt dh co b w -> (dt dh co) b w")
                        if bt == NBT - 1 and g == NG - 1:
                            # final group: per-j quarter DMAs so the drain
                            # tail is one eviction + 0.13 MB, not 0.28 MB.
                            h0 = GB * g + 4 * j
                            nc.sync.dma_start(
                                out=dst[:, h0:h0 + 4, :],
                                in_=out_g[:, 4 * j:4 * j + 4, :])
                        elif j % 2 == 1:
                            # write each 8-row half as soon as its two
                            # evictions land — halves the drain tail.
                            h0 = GB * g + 4 * (j - 1)
                            nc.sync.dma_start(
                                out=dst[:, h0:h0 + 8, :],
                                in_=out_g[:, 4 * (j - 1):4 * (j + 1), :])
    if split:
        _split_excess_waits(nc)
    return nc


_NC_CACHE = []


def _get_nc():
    if not _NC_CACHE:
        _NC_CACHE.append(_build_program())
    return _NC_CACHE[0]


def _pack_weights(weight):
    wt = np.zeros((6, 128, 128), np.float32)
    for kw in range(3):
        for jhh in range(2):
            i = kw * 2 + jhh
            for jt in range(4):
                for jhg in range(2):
                    jh = 2 * jhh + jhg
                    r0 = jt * 32 + jhg * 16
                    for dt in range(2):
                        kt = jt - dt
                        if not 0 <= kt < KS:
                            continue
                        for dh in range(2):
                            kh = jh - dh
                            if not 0 <= kh < KS:
                                continue
                            c0 = dt * 64 + dh * 32
                            wt[i, r0:r0 + 16, c0:c0 + 32] = \
                                weight[:, :, kt, kh, kw].T
    # partition-major [p, i, m] so the SBUF load is one big-packet DMA
    return np.ascontiguousarray(
        wt.transpose(1, 0, 2)).astype(ml_dtypes.bfloat16)


def run(x, weight, bias, trace=False):
    x = np.asarray(x, dtype=np.float32)
    weight = np.asarray(weight, dtype=np.float32)
    bias = np.asarray(bias, dtype=np.float32)

    xp = np.zeros((B, C_IN, T + 2, H + 2, W + 2), ml_dtypes.bfloat16)
    xp[:, :, 1:-1, 1:-1, 1:-1] = x.astype(ml_dtypes.bfloat16)
    wt = _pack_weights(weight)
    bi = np.tile(bias, 4).reshape(128, 1).astype(np.float32)

    in_maps = []
    for c in range(N_CORES):
        b, q = divmod(c, 4)
        t0 = q * TSH
        sh = xp[b, :, t0:t0 + TSH + 2]                # [ci, f, 130, 130]
        sh = sh.reshape(C_IN, TSH + 2, HB + 1, 2, W + 2)
        sh = np.ascontiguousarray(sh.transpose(1, 3, 0, 2, 4))
        in_maps.append({"xs": sh, "wt": wt, "bi": bi})

    nc = _get_nc()
    res = run_bass_kernel_spmd(nc, in_maps, list(range(N_CORES)), trace=trace)

    outp = np.empty((B, C_OUT, T, H, W), np.float32)
    for c in range(N_CORES):
        b, q = divmod(c, 4)
        r = res.results[c]["out"]                     # [bt, dt, dh, co, bh, w]
        r = np.asarray(r).astype(np.float32)
        r = r.transpose(3, 0, 1, 4, 2, 5).reshape(C_OUT, TSH, H, W)
        outp[b, :, q * TSH:(q + 1) * TSH] = r
    return outp, res


def kernel(x, weight, bias):
    outp, _ = run(x, weight, bias, trace=False)
    return outp
